# revision 21
# baseline (speedup 1.0000x reference)
"""Bass/Trainium2 kernel for nn_Attn_70076686401576 (block-causal-biased MHA).

Math (per reference):
  qkv = x @ Wqkv + bqkv  -> split into q,k,v heads (H=16, hd=64)
  q,k RMS-normalized over head dim (QKNorm, eps=1e-6, scales gq/gk)
  scores = q k^T / sqrt(hd) + M, where M[i,j] = 1.0 for future-frame keys
  attn = softmax(scores); o = attn @ v; out = o @ Wout + bout
Sharding: 16 heads / 8 cores = 2 heads per core (head-parallel).  Each core
computes its 2 heads' q/k/v from the full x (Wqkv column-sharded), runs full
attention for those heads, and produces a partial output via the row-sharded
Wout.  Host sums the 8 partials (+ bout).

Key structure (v2):
  - phase 1: x tiles DMA'd f32, cast to bf16 on GpSimd, transposed on PE
    (grouped 4-at-a-time into PSUM, copied out alternately by Scalar/Vector),
    projected with bf16 weights; QKNorm folds the 1/sqrt(hd) score scale and
    the gq/gk scales into the Sqrt activation's per-partition scale/bias.
  - phase 2 is a per-ktile software pipeline: the two heads' score matmuls
    run concurrently on PE row-halves into adjacent PSUM banks; the combined
    [128,2,512] tile is exponentiated by ONE engine op, alternating between
    ScalarE (exact ACT Exp) and DVE (Schraudolph bit-trick exp: one affine
    tensor_scalar emitting int16 bf16-bit-patterns) so the two engines share
    the exp load; attn@V accumulates into per-stripe PSUM; the "+1.0
    future-frame" mask is folded into the exp (ACT bias=+1 / Schraudolph
    +A_SCH) so no e-scaled V copy is needed.
  - softmax denominator via a ones-column appended to V; normalization via
    reciprocal + PE row-broadcast; output projection from bf16 oTn.
"""

import math
import numpy as np

N_TOK_FULL = 4096
D_MODEL = 1024
HD = 64
TPF = 256
EPS = 1e-6
N_CORES = 8

LN2 = math.log(2.0)
SCH_A = 128.0 / LN2            # bf16 Schraudolph multiplier
SCH_C = -6.0                   # balance constant (minimizes max rel err)
SCH_B = 127.0 * 128.0 + SCH_C


def build_program(n_tok=N_TOK_FULL, debug=False):
    import concourse.bass as bass
    import concourse.tile as tile
    from concourse import bacc, mybir
    from concourse.masks import make_identity
    from contextlib import ExitStack

    f32 = mybir.dt.float32
    f32r = mybir.dt.float32r
    bf16 = mybir.dt.bfloat16
    i16 = mybir.dt.int16
    AF = mybir.ActivationFunctionType
    MUL = mybir.AluOpType.mult
    ADD = mybir.AluOpType.add

    D = D_MODEL
    n_ranges = n_tok // 512
    n_ktiles = n_tok // 128
    n_stripes = n_tok // 512

    nc = bacc.Bacc("TRN2", target_bir_lowering=False, debug=False,
                   num_devices=N_CORES)
    x_d = nc.dram_tensor("x", [n_tok, D], f32, kind="ExternalInput").ap()
    wqkv_d = nc.dram_tensor("wqkv", [D, 384], f32, kind="ExternalInput").ap()
    bqkv_d = nc.dram_tensor("bqkv", [384], f32, kind="ExternalInput").ap()
    # gv: per-partition [scale_q, bias_q, scale_k, bias_k] for the RMS sqrt
    gv_d = nc.dram_tensor("gv", [128, 4], f32, kind="ExternalInput").ap()
    wout_d = nc.dram_tensor("wout", [128, D], f32, kind="ExternalInput").ap()
    out_d = nc.dram_tensor("out", [n_tok, D], f32, kind="ExternalOutput").ap()

    x_t = x_d.rearrange("(t p) d -> t p d", p=128)
    out_t = out_d.rearrange("(t p) d -> t p d", p=128)

    with tile.TileContext(nc) as tc:
        ctx = ExitStack()
        sb = ctx.enter_context(tc.tile_pool(name="sb", bufs=1))
        ps1_ctx = ExitStack()
        ps1 = ps1_ctx.enter_context(
            tc.tile_pool(name="ps1", bufs=1, space="PSUM"))
        sbp_ctx = ExitStack()
        sbp = sbp_ctx.enter_context(tc.tile_pool(name="sbp", bufs=1))

        # ---- constants ----
        identf = sb.tile([128, 128], f32, tag="identf")
        make_identity(nc, identf)
        identb = sb.tile([128, 128], bf16, tag="identb")
        nc.vector.tensor_copy(identb, identf)
        # block-diag ones: blkdiag.T @ sq -> per-head column sums broadcast
        # to that head's 64 partitions
        blkdf = sb.tile([128, 128], f32, tag="blkdf")
        nc.gpsimd.memset(blkdf, 0.0)
        nc.gpsimd.memset(blkdf[0:64, 0:64], 1.0)
        nc.gpsimd.memset(blkdf[64:128, 64:128], 1.0)
        blkdiag = sb.tile([128, 128], f32r, tag="blkdiag")
        nc.vector.tensor_copy(blkdiag, blkdf)
        ones64 = sb.tile([128, 64], f32, tag="ones64")
        nc.gpsimd.memset(ones64, 1.0)

        wqkvf = sbp.tile([128, 8, 384], f32, tag="wqkvf")
        nc.sync.dma_start(wqkvf, wqkv_d.rearrange("(c p) n -> p c n", p=128))
        wqkv_sb = sb.tile([128, 8, 384], bf16, tag="wqkv")
        nc.vector.tensor_copy(wqkv_sb, wqkvf)
        bq_sb = sb.tile([128, 3], f32, tag="bq")
        nc.sync.dma_start(bq_sb, bqkv_d.rearrange("(c p) -> p c", p=128))
        gv_sb = sb.tile([128, 4], f32, tag="gv")
        nc.sync.dma_start(gv_sb, gv_d)
        wof = sb.tile([128, D], f32, tag="wof")
        nc.sync.dma_start(wof, wout_d)
        wo0 = sb.tile([64, D], bf16, tag="wo0")
        nc.vector.tensor_copy(wo0, wof[0:64, :])
        wo1 = sb.tile([64, D], bf16, tag="wo1")
        nc.vector.tensor_copy(wo1, wof[64:128, :])

        # ---- persistent blocks ----
        qTb = sb.tile([128, n_tok], bf16, tag="qTb")   # normalized q^T
        kTb = sb.tile([128, n_tok], bf16, tag="kTb")
        oTn0 = sb.tile([64, n_tok], bf16, tag="oTn0")
        oTn1 = sb.tile([64, n_tok], bf16, tag="oTn1")
        # V natural layout per (ktile, head): [keys=128, kt, h, hd+ones]
        va = sb.tile([128, n_ktiles, 2, 65], bf16, tag="va")
        nc.gpsimd.memset(va[:, :, :, 64:65], 1.0)

        # ================= phase 1: projection + QKNorm =================
        # x tiles: DMA f32 -> cast bf16 (Scalar/Vector alternating) -> DMA
        # xbar transpose (SBUF->SBUF, zero PE) into xT; projection per range
        # interleaves as soon as its 4 tiles are transposed.
        # xbar mapping (HW-verified): out[p, m, c] = in[c, m*128 + p], i.e.
        # xT[d % 128, d // 128, tok] = x[tok, d] -- matches wqkv_sb layout.
        xT = sb.tile([128, 8, n_tok], bf16, tag="xT")

        def emit_proj_range(r):
            pj = ps1.tile([128, 3, 512], f32, tag="pj", bufs=1, name=f"pj_{r}")
            for dc in range(8):
                for oc in range(3):
                    nc.tensor.matmul(
                        pj[:, oc, :],
                        wqkv_sb[:, dc, oc * 128:(oc + 1) * 128],
                        xT[:, dc, r * 512:(r + 1) * 512],
                        start=(dc == 0), stop=(dc == 7))
            sl = slice(r * 512, (r + 1) * 512)
            qTr = sbp.tile([128, 512], f32r, tag="qTr", bufs=2)
            kTr = sbp.tile([128, 512], f32r, tag="kTr", bufs=2)
            vTr = sbp.tile([128, 512], bf16, tag="vTr", bufs=2)
            nc.vector.tensor_scalar_add(qTr, pj[:, 0, :], bq_sb[:, 0:1])
            nc.vector.tensor_scalar_add(kTr, pj[:, 1, :], bq_sb[:, 1:2])
            nc.vector.tensor_scalar_add(vTr, pj[:, 2, :], bq_sb[:, 2:3])

            # QKNorm: rs = g * rsqrt(mean(t^2) + eps); q additionally folds
            # the 1/sqrt(hd) score scale.  scale/bias of the Sqrt activation
            # are per-partition host-precomputed: sqs = sqrt(sumsq*s + b),
            # rs = 1/sqs.
            for blk, blkb, gcol in ((qTr, qTb, 0), (kTr, kTb, 1)):
                sq = sbp.tile([128, 512], f32r, tag="sq", bufs=2)
                nc.scalar.activation(sq, blk, AF.Square)
                ps_r = ps1.tile([128, 512], f32, tag="psr", bufs=2,
                                name=f"psr_{r}_{gcol}")
                nc.tensor.matmul(ps_r, blkdiag, sq, start=True, stop=True)
                sqs = sbp.tile([128, 512], f32, tag="sqs", bufs=2)
                nc.scalar.activation(sqs, ps_r, AF.Sqrt,
                                     bias=gv_sb[:, 2 * gcol + 1:2 * gcol + 2],
                                     scale=gv_sb[:, 2 * gcol:2 * gcol + 1])
                rs = sbp.tile([128, 512], f32, tag="rs", bufs=2)
                nc.vector.reciprocal_approx_fast(rs, sqs)
                nc.vector.tensor_mul(blkb[:, sl], blk, rs)

            # V -> va for this range's 4 ktiles
            vp = ps1.tile([128, 8, 128], bf16, tag="xp", bufs=2,
                          name=f"vp_{r}")
            for q in range(4):
                nc.tensor.transpose(
                    vp[:, q, :], vTr[:, q * 128:(q + 1) * 128], identb)
            src = vp[:, 0:4, :].rearrange("p k (h d) -> p k h d", h=2)
            if r % 2 == 0:
                nc.scalar.copy(va[:, 4 * r:4 * r + 4, :, 0:64], src)
            else:
                nc.vector.tensor_copy(va[:, 4 * r:4 * r + 4, :, 0:64], src)

        # in-DMAs stay on the Sync queue (never blocked); xbar-transpose
        # triggers all go on the Scalar queue right after that tile's cast,
        # so no trigger ever stalls a queue carrying in-DMA triggers.
        for gt in range(n_tok // 128):
            xinf = sbp.tile([128, D], f32, tag="xinf", bufs=4)
            nc.sync.dma_start(xinf[0:64, :], x_t[gt][0:64, :])
            nc.sync.dma_start(xinf[64:128, :], x_t[gt][64:128, :])
            xin = sbp.tile([128, D], bf16, tag="xin", bufs=4)
            if gt % 2 == 0:
                nc.scalar.copy(xin, xinf)
            else:
                nc.vector.tensor_copy(xin, xinf)
            nc.scalar.dma_start(xT[:, :, gt * 128:(gt + 1) * 128], xin,
                                transpose=True)
            if gt % 4 == 3:
                emit_proj_range(gt // 4)

        # ================= phase 2: attention =================
        sbp_ctx.close()
        ps1_ctx.close()
        ps2_ctx = ExitStack()
        ps2 = ps2_ctx.enter_context(
            tc.tile_pool(name="ps2", bufs=1, space="PSUM"))
        sba_ctx = ExitStack()
        sba = sba_ctx.enter_context(tc.tile_pool(name="sba", bufs=1))

        # raw per-stripe accumulators (numerator rows 0:64, denominator row
        # 64), flushed from PSUM right after each stripe; normalization +
        # output projection read oAcc (SBUF), so they impose no PSUM WAR on
        # the next stripe and can be spread through its kt loop.
        oAcc = sba.tile([65, 2, n_tok], f32, tag="oAcc")

        def make_norm_steps(s):
            """Normalize stripe s + output proj as spread emission steps."""
            qsl = slice(s * 512, (s + 1) * 512)
            st = {}

            def step_recip():
                rd = sba.tile([65, 2, 512], f32, tag="rd", bufs=2,
                              name=f"rd_{s}")
                nc.vector.reciprocal_approx_fast(rd, oAcc[:, :, qsl])
                st["rd"] = rd

            def step_bcast():
                ps_b = ps2.tile([128, 2, 512], f32, tag="sg", bufs=3,
                                name=f"psb_{s}")
                for h in range(2):
                    nc.tensor.matmul(ps_b[0:64, h, :],
                                     ones64[64:65, :],
                                     st["rd"][64:65, h, :],
                                     start=True, stop=True,
                                     tile_position=(64, 0))
                st["psb"] = ps_b

            def step_rb():
                rb = sba.tile([64, 2, 512], f32, tag="rb", bufs=2,
                              name=f"rb_{s}")
                nc.scalar.copy(rb, st["psb"][0:64, :, :])
                st["rb"] = rb

            def step_mul():
                for h, oTn in ((0, oTn0), (1, oTn1)):
                    nc.vector.tensor_mul(oTn[:, qsl], oAcc[0:64, h, qsl],
                                         st["rb"][:, h, :])

            def make_proj(tt):
                def step_proj():
                    t0 = s * 512 + tt * 128
                    ps_o = ps2.tile([128, 2, 512], f32, tag="sg", bufs=3,
                                    name=f"pso_{s}_{tt}")
                    for half in range(2):
                        nsl = slice(half * 512, (half + 1) * 512)
                        nc.tensor.matmul(ps_o[:, half, :],
                                         oTn0[:, t0:t0 + 128],
                                         wo0[:, nsl],
                                         start=True, stop=False)
                        nc.tensor.matmul(ps_o[:, half, :],
                                         oTn1[:, t0:t0 + 128],
                                         wo1[:, nsl],
                                         start=False, stop=True)
                    ob = sba.tile([128, 2, 512], f32, tag="ob", bufs=3,
                                  name=f"ob_{s}_{tt}")
                    if tt % 2 == 0:
                        nc.scalar.copy(ob, ps_o)
                    else:
                        nc.vector.tensor_copy(ob, ps_o)
                    nc.sync.dma_start(out_t[t0 // 128], ob)
                return step_proj

            return [step_recip, step_bcast, step_rb, step_mul,
                    make_proj(0), make_proj(1), make_proj(2), make_proj(3)]

        # norm steps of stripe s-1 emitted at these kt indices of stripe s
        STEP_KTS = {2: 0, 5: 1, 8: 2, 11: 3, 14: 4, 18: 5, 22: 6, 26: 7}

        LAG = 2  # attn@V trails scores/exp by LAG ktiles so PE never waits
        pending = None
        for s in range(n_stripes):
            qsl = slice(s * 512, (s + 1) * 512)
            po = ps2.tile([65, 2, 512], f32, tag="po", bufs=1,
                          name=f"po_{s}")
            prev = []
            for kt in range(n_ktiles):
                sg = ps2.tile([128, 2, 512], f32, tag="sg", bufs=3,
                              name=f"sg_{s}_{kt}")
                for h in range(2):
                    hp = slice(h * 64, (h + 1) * 64)
                    nc.tensor.matmul(
                        sg[:, h, :],
                        kTb[hp, kt * 128:(kt + 1) * 128],
                        qTb[hp, qsl],
                        start=True, stop=True,
                        tile_position=(h * 64, 0))
                # exp (mask folded in): key frame fk vs query frames
                # (2s, 2s+1): future -> +1.0 bias; whole [128,2,512] tile by
                # one engine, alternating ScalarE (exact) / DVE (Schraudolph)
                et = sba.tile([128, 2, 512], bf16, tag="et", bufs=5,
                              name=f"et_{s}_{kt}")
                fk = kt // 2
                if kt % 2 == 0:
                    if fk == 2 * s + 1:
                        nc.scalar.activation(et[:, :, 0:256],
                                             sg[:, :, 0:256], AF.Exp,
                                             bias=1.0)
                        nc.scalar.activation(et[:, :, 256:512],
                                             sg[:, :, 256:512], AF.Exp)
                    else:
                        nc.scalar.activation(
                            et, sg, AF.Exp,
                            bias=(1.0 if fk > 2 * s + 1 else 0.0))
                else:
                    eti = et.bitcast(i16)
                    if fk == 2 * s + 1:
                        nc.vector.tensor_scalar(
                            eti[:, :, 0:256], sg[:, :, 0:256],
                            SCH_A, SCH_B + SCH_A, op0=MUL, op1=ADD)
                        nc.vector.tensor_scalar(
                            eti[:, :, 256:512], sg[:, :, 256:512],
                            SCH_A, SCH_B, op0=MUL, op1=ADD)
                    else:
                        b = SCH_B + (SCH_A if fk > 2 * s + 1 else 0.0)
                        nc.vector.tensor_scalar(
                            eti, sg, SCH_A, b, op0=MUL, op1=ADD)
                if pending is not None and kt in STEP_KTS:
                    pending[STEP_KTS[kt]]()
                    if STEP_KTS[kt] == len(pending) - 1:
                        pending = None
                prev.append((kt, et))
                if len(prev) > LAG:
                    pkt, pet = prev.pop(0)
                    for h in range(2):
                        nc.tensor.matmul(po[:, h, :],
                                         va[:, pkt, h, :],
                                         pet[:, h, :],
                                         start=(pkt == 0), stop=False)
            for pkt, pet in prev:
                for h in range(2):
                    nc.tensor.matmul(po[:, h, :], va[:, pkt, h, :],
                                     pet[:, h, :],
                                     start=False, stop=(pkt == n_ktiles - 1))
            prev = []
            # flush raw accumulator to SBUF (frees po for the next stripe)
            if s % 2 == 0:
                nc.scalar.copy(oAcc[:, :, qsl], po)
            else:
                nc.vector.tensor_copy(oAcc[:, :, qsl], po)
            pending = make_norm_steps(s)
        for step in pending:
            step()

        sba_ctx.close()
        ps2_ctx.close()
        ctx.close()

    nc.compile()
    return nc


def shard_inputs(x, Wqkv, bqkv, gq, gk, Wout, n_tok):
    """Build the 8 per-core input maps (head-parallel sharding)."""
    D = D_MODEL
    in_maps = []
    gq = np.asarray(gq, np.float64)
    gk = np.asarray(gk, np.float64)
    # per-partition RMS sqrt scale/bias (see build_program):
    #   q: rs = gq/8 * rsqrt(mean+eps)  -> sqs = sqrt(sumsq/gq^2 + 64eps/gq^2)
    #   k: rs = gk * rsqrt(mean+eps)    -> sqs = sqrt(sumsq/(64gk^2) + eps/gk^2)
    sq_ = np.concatenate([1.0 / gq**2, 1.0 / gq**2])
    bq_ = np.concatenate([64.0 * EPS / gq**2, 64.0 * EPS / gq**2])
    sk_ = np.concatenate([1.0 / (64.0 * gk**2), 1.0 / (64.0 * gk**2)])
    bk_ = np.concatenate([EPS / gk**2, EPS / gk**2])
    gv = np.stack([sq_, bq_, sk_, bk_], axis=1).astype(np.float32)
    for c in range(N_CORES):
        cs = slice(128 * c, 128 * (c + 1))
        wq = Wqkv[:, cs]
        wk = Wqkv[:, D + 128 * c:D + 128 * (c + 1)]
        wv = Wqkv[:, 2 * D + 128 * c:2 * D + 128 * (c + 1)]
        wqkv_s = np.ascontiguousarray(np.concatenate([wq, wk, wv], axis=1),
                                      dtype=np.float32)
        bq = bqkv[cs]
        bk = bqkv[D + 128 * c:D + 128 * (c + 1)]
        bv = bqkv[2 * D + 128 * c:2 * D + 128 * (c + 1)]
        bqkv_s = np.ascontiguousarray(np.concatenate([bq, bk, bv]),
                                      dtype=np.float32)
        wout_s = np.ascontiguousarray(Wout[cs, :], dtype=np.float32)
        in_maps.append({
            "x": np.ascontiguousarray(x[:n_tok], dtype=np.float32),
            "wqkv": wqkv_s,
            "bqkv": bqkv_s,
            "gv": np.ascontiguousarray(gv),
            "wout": wout_s,
        })
    return in_maps


_PROGRAM_CACHE = {}


def _get_program(n_tok):
    if n_tok not in _PROGRAM_CACHE:
        _PROGRAM_CACHE[n_tok] = build_program(n_tok)
    return _PROGRAM_CACHE[n_tok]


def run_sharded(inputs, trace=False, tmpdir=None):
    """Run the SPMD kernel; returns (full_output [1,N,D], BassKernelResults)."""
    from concourse.bass_utils import run_bass_kernel_spmd

    x = np.asarray(inputs["x"], dtype=np.float32)
    Wqkv = np.asarray(inputs["Wqkv"], dtype=np.float32)
    bqkv = np.asarray(inputs["bqkv"], dtype=np.float32)
    Wout = np.asarray(inputs["Wout"], dtype=np.float32)
    bout = np.asarray(inputs["bout"], dtype=np.float32)
    gq = np.asarray(inputs["gq"], dtype=np.float32)
    gk = np.asarray(inputs["gk"], dtype=np.float32)
    tpf = int(np.asarray(inputs["tokens_per_frame"]))
    assert tpf == TPF, f"kernel hardcodes tokens_per_frame={TPF}, got {tpf}"

    B, N, D = x.shape
    assert B == 1 and D == D_MODEL
    x2 = x[0]

    nc = _get_program(N)
    in_maps = shard_inputs(x2, Wqkv, bqkv, gq, gk, Wout, N)
    res = run_bass_kernel_spmd(nc, in_maps, list(range(N_CORES)),
                               trace=trace, tmpdir=tmpdir)
    acc = res.results[0]["out"].astype(np.float32)
    for c in range(1, N_CORES):
        acc = acc + res.results[c]["out"]
    if np.any(bout):
        acc = acc + bout[None, :]
    return acc[None], res


def kernel(**inputs):
    out, _ = run_sharded(inputs)
    return out


# revision 23
# speedup vs baseline: 1.1331x; 1.1331x over previous
"""Bass/Trainium2 kernel for nn_Attn_70076686401576 (block-causal-biased MHA).

Math (per reference):
  qkv = x @ Wqkv + bqkv  -> split into q,k,v heads (H=16, hd=64)
  q,k RMS-normalized over head dim (QKNorm, eps=1e-6, scales gq/gk)
  scores = q k^T / sqrt(hd) + M, where M[i,j] = 1.0 for future-frame keys
  attn = softmax(scores); o = attn @ v; out = o @ Wout + bout
Sharding: 16 heads / 8 cores = 2 heads per core (head-parallel).  Each core
computes its 2 heads' q/k/v from the full x (Wqkv column-sharded), runs full
attention for those heads, and produces a partial output via the row-sharded
Wout.  Host sums the 8 partials (+ bout).

Key structure (v2):
  - phase 1: x tiles DMA'd f32, cast to bf16 on GpSimd, transposed on PE
    (grouped 4-at-a-time into PSUM, copied out alternately by Scalar/Vector),
    projected with bf16 weights; QKNorm folds the 1/sqrt(hd) score scale and
    the gq/gk scales into the Sqrt activation's per-partition scale/bias.
  - phase 2 is a per-ktile software pipeline: the two heads' score matmuls
    run concurrently on PE row-halves into adjacent PSUM banks; the combined
    [128,2,512] tile is exponentiated by ONE engine op, alternating between
    ScalarE (exact ACT Exp) and DVE (Schraudolph bit-trick exp: one affine
    tensor_scalar emitting int16 bf16-bit-patterns) so the two engines share
    the exp load; attn@V accumulates into per-stripe PSUM; the "+1.0
    future-frame" mask is folded into the exp (ACT bias=+1 / Schraudolph
    +A_SCH) so no e-scaled V copy is needed.
  - softmax denominator via a ones-column appended to V; normalization via
    reciprocal + PE row-broadcast; output projection from bf16 oTn.
"""

import math
import numpy as np

N_TOK_FULL = 4096
D_MODEL = 1024
HD = 64
TPF = 256
EPS = 1e-6
N_CORES = 8

LN2 = math.log(2.0)
SCH_A = 128.0 / LN2            # bf16 Schraudolph multiplier
SCH_C = -6.0                   # balance constant (minimizes max rel err)
SCH_B = 127.0 * 128.0 + SCH_C


def build_program(n_tok=N_TOK_FULL, debug=False):
    import concourse.bass as bass
    import concourse.tile as tile
    from concourse import bacc, mybir
    from concourse.masks import make_identity
    from contextlib import ExitStack

    f32 = mybir.dt.float32
    f32r = mybir.dt.float32r
    bf16 = mybir.dt.bfloat16
    i16 = mybir.dt.int16
    AF = mybir.ActivationFunctionType
    MUL = mybir.AluOpType.mult
    ADD = mybir.AluOpType.add

    D = D_MODEL
    n_ranges = n_tok // 512
    n_ktiles = n_tok // 128
    n_stripes = n_tok // 512

    nc = bacc.Bacc("TRN2", target_bir_lowering=False, debug=False,
                   num_devices=N_CORES)
    x_d = nc.dram_tensor("x", [n_tok, D], f32, kind="ExternalInput").ap()
    wqkv_d = nc.dram_tensor("wqkv", [D, 384], f32, kind="ExternalInput").ap()
    bqkv_d = nc.dram_tensor("bqkv", [384], f32, kind="ExternalInput").ap()
    # gv: per-partition [scale_q, bias_q, scale_k, bias_k] for the RMS sqrt
    gv_d = nc.dram_tensor("gv", [128, 4], f32, kind="ExternalInput").ap()
    wout_d = nc.dram_tensor("wout", [128, D], f32, kind="ExternalInput").ap()
    out_d = nc.dram_tensor("out", [n_tok, D], f32, kind="ExternalOutput").ap()

    x_t = x_d.rearrange("(t p) d -> t p d", p=128)
    out_t = out_d.rearrange("(t p) d -> t p d", p=128)

    with tile.TileContext(nc) as tc:
        ctx = ExitStack()
        sb = ctx.enter_context(tc.tile_pool(name="sb", bufs=1))
        ps1_ctx = ExitStack()
        ps1 = ps1_ctx.enter_context(
            tc.tile_pool(name="ps1", bufs=1, space="PSUM"))
        sbp_ctx = ExitStack()
        sbp = sbp_ctx.enter_context(tc.tile_pool(name="sbp", bufs=1))

        # ---- constants ----
        identf = sb.tile([128, 128], f32, tag="identf")
        make_identity(nc, identf)
        identb = sb.tile([128, 128], bf16, tag="identb")
        nc.vector.tensor_copy(identb, identf)
        # block-diag ones: blkdiag.T @ sq -> per-head column sums broadcast
        # to that head's 64 partitions
        blkdf = sb.tile([128, 128], f32, tag="blkdf")
        nc.gpsimd.memset(blkdf, 0.0)
        nc.gpsimd.memset(blkdf[0:64, 0:64], 1.0)
        nc.gpsimd.memset(blkdf[64:128, 64:128], 1.0)
        blkdiag = sb.tile([128, 128], f32r, tag="blkdiag")
        nc.vector.tensor_copy(blkdiag, blkdf)
        ones64 = sb.tile([128, 64], f32, tag="ones64")
        nc.gpsimd.memset(ones64, 1.0)

        wqkvf = sbp.tile([128, 8, 384], f32, tag="wqkvf")
        nc.sync.dma_start(wqkvf, wqkv_d.rearrange("(c p) n -> p c n", p=128))
        wqkv_sb = sb.tile([128, 8, 384], bf16, tag="wqkv")
        nc.vector.tensor_copy(wqkv_sb, wqkvf)
        bq_sb = sb.tile([128, 3], f32, tag="bq")
        nc.sync.dma_start(bq_sb, bqkv_d.rearrange("(c p) -> p c", p=128))
        gv_sb = sb.tile([128, 4], f32, tag="gv")
        nc.sync.dma_start(gv_sb, gv_d)
        wof = sb.tile([128, D], f32, tag="wof")
        nc.sync.dma_start(wof, wout_d)
        wo0 = sb.tile([64, D], bf16, tag="wo0")
        nc.vector.tensor_copy(wo0, wof[0:64, :])
        wo1 = sb.tile([64, D], bf16, tag="wo1")
        nc.vector.tensor_copy(wo1, wof[64:128, :])

        # ---- persistent blocks ----
        qTb = sb.tile([128, n_tok], bf16, tag="qTb")   # normalized q^T
        kTb = sb.tile([128, n_tok], bf16, tag="kTb")
        oTn0 = sb.tile([64, n_tok], bf16, tag="oTn0")
        oTn1 = sb.tile([64, n_tok], bf16, tag="oTn1")
        # V natural layout per (ktile, head): [keys=128, kt, h, hd+ones]
        va = sb.tile([128, n_ktiles, 2, 65], bf16, tag="va")
        nc.gpsimd.memset(va[:, :, :, 64:65], 1.0)

        # ================= phase 1: projection + QKNorm =================
        # x tiles: DMA f32 -> cast bf16 (Scalar/Vector alternating) -> DMA
        # xbar transpose (SBUF->SBUF, zero PE) into xT; projection per range
        # interleaves as soon as its 4 tiles are transposed.
        # xbar mapping (HW-verified): out[p, m, c] = in[c, m*128 + p], i.e.
        # xT[d % 128, d // 128, tok] = x[tok, d] -- matches wqkv_sb layout.
        # Per-range ring tiles (NOT one big tile): coarse tile-dependency
        # tracking would otherwise serialize each range's transposes against
        # the previous range's projection reads.

        def emit_proj_range(r, xTr):
            pj = ps1.tile([128, 3, 512], f32, tag="pj", bufs=1, name=f"pj_{r}")
            for dc in range(8):
                for oc in range(3):
                    nc.tensor.matmul(
                        pj[:, oc, :],
                        wqkv_sb[:, dc, oc * 128:(oc + 1) * 128],
                        xTr[:, dc, :],
                        start=(dc == 0), stop=(dc == 7))
            sl = slice(r * 512, (r + 1) * 512)
            qTr = sbp.tile([128, 512], f32r, tag="qTr", bufs=2)
            kTr = sbp.tile([128, 512], f32r, tag="kTr", bufs=2)
            vTr = sbp.tile([128, 512], bf16, tag="vTr", bufs=2)
            nc.vector.tensor_scalar_add(qTr, pj[:, 0, :], bq_sb[:, 0:1])
            nc.vector.tensor_scalar_add(kTr, pj[:, 1, :], bq_sb[:, 1:2])
            nc.vector.tensor_scalar_add(vTr, pj[:, 2, :], bq_sb[:, 2:3])

            # QKNorm: rs = g * rsqrt(mean(t^2) + eps); q additionally folds
            # the 1/sqrt(hd) score scale.  scale/bias of the Sqrt activation
            # are per-partition host-precomputed: sqs = sqrt(sumsq*s + b),
            # rs = 1/sqs.
            for blk, blkb, gcol in ((qTr, qTb, 0), (kTr, kTb, 1)):
                sq = sbp.tile([128, 512], f32r, tag="sq", bufs=2)
                nc.scalar.activation(sq, blk, AF.Square)
                ps_r = ps1.tile([128, 512], f32, tag="psr", bufs=2,
                                name=f"psr_{r}_{gcol}")
                nc.tensor.matmul(ps_r, blkdiag, sq, start=True, stop=True)
                sqs = sbp.tile([128, 512], f32, tag="sqs", bufs=2)
                nc.scalar.activation(sqs, ps_r, AF.Sqrt,
                                     bias=gv_sb[:, 2 * gcol + 1:2 * gcol + 2],
                                     scale=gv_sb[:, 2 * gcol:2 * gcol + 1])
                rs = sbp.tile([128, 512], f32, tag="rs", bufs=2)
                nc.vector.reciprocal_approx_fast(rs, sqs)
                nc.vector.tensor_mul(blkb[:, sl], blk, rs)

            # V -> va for this range's 4 ktiles
            vp = ps1.tile([128, 8, 128], bf16, tag="xp", bufs=2,
                          name=f"vp_{r}")
            for q in range(4):
                nc.tensor.transpose(
                    vp[:, q, :], vTr[:, q * 128:(q + 1) * 128], identb)
            src = vp[:, 0:4, :].rearrange("p k (h d) -> p k h d", h=2)
            if r % 2 == 0:
                nc.scalar.copy(va[:, 4 * r:4 * r + 4, :, 0:64], src)
            else:
                nc.vector.tensor_copy(va[:, 4 * r:4 * r + 4, :, 0:64], src)

        # in-DMAs stay on the Sync queue (never blocked); xbar-transpose
        # triggers all go on the Scalar queue right after that tile's cast,
        # so no trigger ever stalls a queue carrying in-DMA triggers.
        xTr = None
        for gt in range(n_tok // 128):
            r, tt = gt // 4, gt % 4
            if tt == 0:
                xTr = sbp.tile([128, 8, 512], bf16, tag="xT", bufs=2,
                               name=f"xTr_{r}")
            xinf = sbp.tile([128, D], f32, tag="xinf", bufs=4)
            nc.sync.dma_start(xinf[0:64, :], x_t[gt][0:64, :])
            nc.sync.dma_start(xinf[64:128, :], x_t[gt][64:128, :])
            xin = sbp.tile([128, D], bf16, tag="xin", bufs=4)
            if gt % 2 == 0:
                nc.scalar.copy(xin, xinf)
            else:
                nc.vector.tensor_copy(xin, xinf)
            nc.scalar.dma_start(xTr[:, :, tt * 128:(tt + 1) * 128], xin,
                                transpose=True)
            if tt == 3:
                emit_proj_range(r, xTr)

        # ================= phase 2: attention =================
        sbp_ctx.close()
        ps1_ctx.close()
        ps2_ctx = ExitStack()
        ps2 = ps2_ctx.enter_context(
            tc.tile_pool(name="ps2", bufs=1, space="PSUM"))
        sba_ctx = ExitStack()
        sba = sba_ctx.enter_context(tc.tile_pool(name="sba", bufs=1))

        # raw per-stripe accumulators (numerator rows 0:64, denominator row
        # 64), flushed from PSUM right after each stripe; normalization +
        # output projection read oAcc (SBUF), so they impose no PSUM WAR on
        # the next stripe and can be spread through its kt loop.
        oAcc = sba.tile([65, 2, n_tok], f32, tag="oAcc")

        def make_norm_steps(s):
            """Normalize stripe s + output proj as spread emission steps."""
            qsl = slice(s * 512, (s + 1) * 512)
            st = {}

            def step_recip():
                rd = sba.tile([65, 2, 512], f32, tag="rd", bufs=2,
                              name=f"rd_{s}")
                nc.vector.reciprocal_approx_fast(rd, oAcc[:, :, qsl])
                st["rd"] = rd

            def step_bcast():
                ps_b = ps2.tile([128, 2, 512], f32, tag="sg", bufs=3,
                                name=f"psb_{s}")
                for h in range(2):
                    nc.tensor.matmul(ps_b[0:64, h, :],
                                     ones64[64:65, :],
                                     st["rd"][64:65, h, :],
                                     start=True, stop=True,
                                     tile_position=(64, 0))
                st["psb"] = ps_b

            def step_rb():
                rb = sba.tile([64, 2, 512], f32, tag="rb", bufs=2,
                              name=f"rb_{s}")
                nc.scalar.copy(rb, st["psb"][0:64, :, :])
                st["rb"] = rb

            def step_mul():
                for h, oTn in ((0, oTn0), (1, oTn1)):
                    nc.vector.tensor_mul(oTn[:, qsl], oAcc[0:64, h, qsl],
                                         st["rb"][:, h, :])

            def make_proj(tt):
                def step_proj():
                    t0 = s * 512 + tt * 128
                    ps_o = ps2.tile([128, 2, 512], f32, tag="sg", bufs=3,
                                    name=f"pso_{s}_{tt}")
                    for half in range(2):
                        nsl = slice(half * 512, (half + 1) * 512)
                        nc.tensor.matmul(ps_o[:, half, :],
                                         oTn0[:, t0:t0 + 128],
                                         wo0[:, nsl],
                                         start=True, stop=False)
                        nc.tensor.matmul(ps_o[:, half, :],
                                         oTn1[:, t0:t0 + 128],
                                         wo1[:, nsl],
                                         start=False, stop=True)
                    ob = sba.tile([128, 2, 512], f32, tag="ob", bufs=3,
                                  name=f"ob_{s}_{tt}")
                    if tt % 2 == 0:
                        nc.scalar.copy(ob, ps_o)
                    else:
                        nc.vector.tensor_copy(ob, ps_o)
                    nc.sync.dma_start(out_t[t0 // 128], ob)
                return step_proj

            return [step_recip, step_bcast, step_rb, step_mul,
                    make_proj(0), make_proj(1), make_proj(2), make_proj(3)]

        # norm steps of stripe s-1 emitted at these kt indices of stripe s
        STEP_KTS = {2: 0, 5: 1, 8: 2, 11: 3, 14: 4, 18: 5, 22: 6, 26: 7}

        LAG = 2  # attn@V trails scores/exp by LAG ktiles so PE never waits
        pending = None
        for s in range(n_stripes):
            qsl = slice(s * 512, (s + 1) * 512)
            po = ps2.tile([65, 2, 512], f32, tag="po", bufs=1,
                          name=f"po_{s}")
            prev = []
            for kt in range(n_ktiles):
                sg = ps2.tile([128, 2, 512], f32, tag="sg", bufs=3,
                              name=f"sg_{s}_{kt}")
                for h in range(2):
                    hp = slice(h * 64, (h + 1) * 64)
                    nc.tensor.matmul(
                        sg[:, h, :],
                        kTb[hp, kt * 128:(kt + 1) * 128],
                        qTb[hp, qsl],
                        start=True, stop=True,
                        tile_position=(h * 64, 0))
                # exp (mask folded in): key frame fk vs query frames
                # (2s, 2s+1): future -> +1.0 bias; whole [128,2,512] tile by
                # one engine, alternating ScalarE (exact) / DVE (Schraudolph)
                et = sba.tile([128, 2, 512], bf16, tag="et", bufs=5,
                              name=f"et_{s}_{kt}")
                fk = kt // 2
                if kt % 2 == 0:
                    if fk == 2 * s + 1:
                        nc.scalar.activation(et[:, :, 0:256],
                                             sg[:, :, 0:256], AF.Exp,
                                             bias=1.0)
                        nc.scalar.activation(et[:, :, 256:512],
                                             sg[:, :, 256:512], AF.Exp)
                    else:
                        nc.scalar.activation(
                            et, sg, AF.Exp,
                            bias=(1.0 if fk > 2 * s + 1 else 0.0))
                else:
                    eti = et.bitcast(i16)
                    if fk == 2 * s + 1:
                        nc.vector.tensor_scalar(
                            eti[:, :, 0:256], sg[:, :, 0:256],
                            SCH_A, SCH_B + SCH_A, op0=MUL, op1=ADD)
                        nc.vector.tensor_scalar(
                            eti[:, :, 256:512], sg[:, :, 256:512],
                            SCH_A, SCH_B, op0=MUL, op1=ADD)
                    else:
                        b = SCH_B + (SCH_A if fk > 2 * s + 1 else 0.0)
                        nc.vector.tensor_scalar(
                            eti, sg, SCH_A, b, op0=MUL, op1=ADD)
                if pending is not None and kt in STEP_KTS:
                    pending[STEP_KTS[kt]]()
                    if STEP_KTS[kt] == len(pending) - 1:
                        pending = None
                prev.append((kt, et))
                if len(prev) > LAG:
                    pkt, pet = prev.pop(0)
                    for h in range(2):
                        nc.tensor.matmul(po[:, h, :],
                                         va[:, pkt, h, :],
                                         pet[:, h, :],
                                         start=(pkt == 0), stop=False)
            for pkt, pet in prev:
                for h in range(2):
                    nc.tensor.matmul(po[:, h, :], va[:, pkt, h, :],
                                     pet[:, h, :],
                                     start=False, stop=(pkt == n_ktiles - 1))
            prev = []
            # flush raw accumulator to SBUF (frees po for the next stripe)
            if s % 2 == 0:
                nc.scalar.copy(oAcc[:, :, qsl], po)
            else:
                nc.vector.tensor_copy(oAcc[:, :, qsl], po)
            pending = make_norm_steps(s)
        for step in pending:
            step()

        sba_ctx.close()
        ps2_ctx.close()
        ctx.close()

    nc.compile()
    return nc


def shard_inputs(x, Wqkv, bqkv, gq, gk, Wout, n_tok):
    """Build the 8 per-core input maps (head-parallel sharding)."""
    D = D_MODEL
    in_maps = []
    gq = np.asarray(gq, np.float64)
    gk = np.asarray(gk, np.float64)
    # per-partition RMS sqrt scale/bias (see build_program):
    #   q: rs = gq/8 * rsqrt(mean+eps)  -> sqs = sqrt(sumsq/gq^2 + 64eps/gq^2)
    #   k: rs = gk * rsqrt(mean+eps)    -> sqs = sqrt(sumsq/(64gk^2) + eps/gk^2)
    sq_ = np.concatenate([1.0 / gq**2, 1.0 / gq**2])
    bq_ = np.concatenate([64.0 * EPS / gq**2, 64.0 * EPS / gq**2])
    sk_ = np.concatenate([1.0 / (64.0 * gk**2), 1.0 / (64.0 * gk**2)])
    bk_ = np.concatenate([EPS / gk**2, EPS / gk**2])
    gv = np.stack([sq_, bq_, sk_, bk_], axis=1).astype(np.float32)
    for c in range(N_CORES):
        cs = slice(128 * c, 128 * (c + 1))
        wq = Wqkv[:, cs]
        wk = Wqkv[:, D + 128 * c:D + 128 * (c + 1)]
        wv = Wqkv[:, 2 * D + 128 * c:2 * D + 128 * (c + 1)]
        wqkv_s = np.ascontiguousarray(np.concatenate([wq, wk, wv], axis=1),
                                      dtype=np.float32)
        bq = bqkv[cs]
        bk = bqkv[D + 128 * c:D + 128 * (c + 1)]
        bv = bqkv[2 * D + 128 * c:2 * D + 128 * (c + 1)]
        bqkv_s = np.ascontiguousarray(np.concatenate([bq, bk, bv]),
                                      dtype=np.float32)
        wout_s = np.ascontiguousarray(Wout[cs, :], dtype=np.float32)
        in_maps.append({
            "x": np.ascontiguousarray(x[:n_tok], dtype=np.float32),
            "wqkv": wqkv_s,
            "bqkv": bqkv_s,
            "gv": np.ascontiguousarray(gv),
            "wout": wout_s,
        })
    return in_maps


_PROGRAM_CACHE = {}


def _get_program(n_tok):
    if n_tok not in _PROGRAM_CACHE:
        _PROGRAM_CACHE[n_tok] = build_program(n_tok)
    return _PROGRAM_CACHE[n_tok]


def run_sharded(inputs, trace=False, tmpdir=None):
    """Run the SPMD kernel; returns (full_output [1,N,D], BassKernelResults)."""
    from concourse.bass_utils import run_bass_kernel_spmd

    x = np.asarray(inputs["x"], dtype=np.float32)
    Wqkv = np.asarray(inputs["Wqkv"], dtype=np.float32)
    bqkv = np.asarray(inputs["bqkv"], dtype=np.float32)
    Wout = np.asarray(inputs["Wout"], dtype=np.float32)
    bout = np.asarray(inputs["bout"], dtype=np.float32)
    gq = np.asarray(inputs["gq"], dtype=np.float32)
    gk = np.asarray(inputs["gk"], dtype=np.float32)
    tpf = int(np.asarray(inputs["tokens_per_frame"]))
    assert tpf == TPF, f"kernel hardcodes tokens_per_frame={TPF}, got {tpf}"

    B, N, D = x.shape
    assert B == 1 and D == D_MODEL
    x2 = x[0]

    nc = _get_program(N)
    in_maps = shard_inputs(x2, Wqkv, bqkv, gq, gk, Wout, N)
    res = run_bass_kernel_spmd(nc, in_maps, list(range(N_CORES)),
                               trace=trace, tmpdir=tmpdir)
    acc = res.results[0]["out"].astype(np.float32)
    for c in range(1, N_CORES):
        acc = acc + res.results[c]["out"]
    if np.any(bout):
        acc = acc + bout[None, :]
    return acc[None], res


def kernel(**inputs):
    out, _ = run_sharded(inputs)
    return out


# revision 24
# speedup vs baseline: 1.3478x; 1.1894x over previous
"""Bass/Trainium2 kernel for nn_Attn_70076686401576 (block-causal-biased MHA).

Math (per reference):
  qkv = x @ Wqkv + bqkv  -> split into q,k,v heads (H=16, hd=64)
  q,k RMS-normalized over head dim (QKNorm, eps=1e-6, scales gq/gk)
  scores = q k^T / sqrt(hd) + M, where M[i,j] = 1.0 for future-frame keys
  attn = softmax(scores); o = attn @ v; out = o @ Wout + bout
Sharding: 16 heads / 8 cores = 2 heads per core (head-parallel).  Each core
computes its 2 heads' q/k/v from the full x (Wqkv column-sharded), runs full
attention for those heads, and produces a partial output via the row-sharded
Wout.  Host sums the 8 partials (+ bout).

Key structure:
  - phase 1 stage A: x tiles DMA'd f32, cast bf16 (Scalar/Vector
    alternating), transposed on PE into one big xT; stage B: PE-dense
    projection + QKNorm (gq/gk and the 1/sqrt(hd) scale folded into the
    Sqrt activation's per-partition scale/bias) + V prep.
  - phase 2 per-ktile software pipeline: both heads' score matmuls run
    concurrently on PE row-halves into adjacent PSUM banks; the combined
    [128,2,512] tile is exponentiated by ONE engine op alternating between
    ScalarE (exact ACT Exp) and DVE (Schraudolph bit-trick: one affine
    tensor_scalar emitting int16 bf16-bit-patterns); the "+1.0 future-frame"
    mask is folded into the exp constants; attn@V trails by LAG ktiles so
    PE never waits on exp latency; po is double-buffered so stripes chain
    without PE gaps; normalization + output projection of stripe s-1 are
    spread through stripe s's loop as emission steps.
  - softmax denominator via a ones-column appended to V; normalization via
    reciprocal + PE row-broadcast.
"""

import math
import numpy as np

N_TOK_FULL = 4096
D_MODEL = 1024
HD = 64
TPF = 256
EPS = 1e-6
N_CORES = 8

LN2 = math.log(2.0)
SCH_A = 128.0 / LN2            # bf16 Schraudolph multiplier
SCH_C = -6.0                   # balance constant (minimizes max rel err)
SCH_B = 127.0 * 128.0 + SCH_C


def build_program(n_tok=N_TOK_FULL, debug=False):
    import concourse.bass as bass
    import concourse.tile as tile
    from concourse import bacc, mybir
    from concourse.masks import make_identity
    from contextlib import ExitStack

    f32 = mybir.dt.float32
    f32r = mybir.dt.float32r
    bf16 = mybir.dt.bfloat16
    i16 = mybir.dt.int16
    AF = mybir.ActivationFunctionType
    MUL = mybir.AluOpType.mult
    ADD = mybir.AluOpType.add

    D = D_MODEL
    n_ranges = n_tok // 512
    n_ktiles = n_tok // 128
    n_stripes = n_tok // 512

    nc = bacc.Bacc("TRN2", target_bir_lowering=False, debug=False,
                   num_devices=N_CORES)
    x_d = nc.dram_tensor("x", [n_tok, D], f32, kind="ExternalInput").ap()
    wqkv_d = nc.dram_tensor("wqkv", [D, 384], f32, kind="ExternalInput").ap()
    bqkv_d = nc.dram_tensor("bqkv", [384], f32, kind="ExternalInput").ap()
    # gv: per-partition [scale_q, bias_q, scale_k, bias_k] for the RMS sqrt
    gv_d = nc.dram_tensor("gv", [128, 4], f32, kind="ExternalInput").ap()
    wout_d = nc.dram_tensor("wout", [128, D], f32, kind="ExternalInput").ap()
    out_d = nc.dram_tensor("out", [n_tok, D], f32, kind="ExternalOutput").ap()

    x_t = x_d.rearrange("(t p) d -> t p d", p=128)
    out_t = out_d.rearrange("(t p) d -> t p d", p=128)

    with tile.TileContext(nc) as tc:
        ctx = ExitStack()
        sb = ctx.enter_context(tc.tile_pool(name="sb", bufs=1))
        ps1_ctx = ExitStack()
        ps1 = ps1_ctx.enter_context(
            tc.tile_pool(name="ps1", bufs=1, space="PSUM"))
        sbp_ctx = ExitStack()
        sbp = sbp_ctx.enter_context(tc.tile_pool(name="sbp", bufs=1))

        # ---- constants ----
        identf = sb.tile([128, 128], f32, tag="identf")
        make_identity(nc, identf)
        identb = sb.tile([128, 128], bf16, tag="identb")
        nc.vector.tensor_copy(identb, identf)
        # block-diag ones: blkdiag.T @ sq -> per-head column sums broadcast
        # to that head's 64 partitions
        blkdf = sb.tile([128, 128], f32, tag="blkdf")
        nc.gpsimd.memset(blkdf, 0.0)
        nc.gpsimd.memset(blkdf[0:64, 0:64], 1.0)
        nc.gpsimd.memset(blkdf[64:128, 64:128], 1.0)
        blkdiag = sb.tile([128, 128], f32r, tag="blkdiag")
        nc.vector.tensor_copy(blkdiag, blkdf)
        ones64 = sb.tile([128, 64], f32, tag="ones64")
        nc.gpsimd.memset(ones64, 1.0)

        wqkvf = sbp.tile([128, 8, 384], f32, tag="wqkvf")
        nc.sync.dma_start(wqkvf, wqkv_d.rearrange("(c p) n -> p c n", p=128))
        wqkv_sb = sb.tile([128, 8, 384], bf16, tag="wqkv")
        nc.vector.tensor_copy(wqkv_sb, wqkvf)
        bq_sb = sb.tile([128, 3], f32, tag="bq")
        nc.sync.dma_start(bq_sb, bqkv_d.rearrange("(c p) -> p c", p=128))
        gv_sb = sb.tile([128, 4], f32, tag="gv")
        nc.sync.dma_start(gv_sb, gv_d)
        wof = sbp.tile([128, D], f32, tag="wof")
        nc.sync.dma_start(wof, wout_d)
        wo0 = sb.tile([64, D], bf16, tag="wo0")
        nc.vector.tensor_copy(wo0, wof[0:64, :])
        wo1 = sb.tile([64, D], bf16, tag="wo1")
        nc.vector.tensor_copy(wo1, wof[64:128, :])

        # ---- persistent blocks ----
        qTb = sb.tile([128, n_tok], bf16, tag="qTb")   # normalized q^T
        kTb = sb.tile([128, n_tok], bf16, tag="kTb")
        oTn0 = sb.tile([64, n_tok], bf16, tag="oTn0")
        oTn1 = sb.tile([64, n_tok], bf16, tag="oTn1")
        # V natural layout per (ktile, head): [keys=128, kt, h, hd+ones]
        va = sb.tile([128, n_ktiles, 2, 65], bf16, tag="va")
        nc.gpsimd.memset(va[:, :, :, 64:65], 1.0)

        # ================= phase 1: projection + QKNorm =================
        # stage A: transpose ALL of x into xT (bf16), DMA-overlapped
        xT = sb.tile([128, 8, n_tok], bf16, tag="xT")
        for gt in range(n_tok // 128):
            xinf = sbp.tile([128, D], f32, tag="xinf", bufs=4)
            nc.sync.dma_start(xinf[0:64, :], x_t[gt][0:64, :])
            nc.sync.dma_start(xinf[64:128, :], x_t[gt][64:128, :])
            xin = sbp.tile([128, D], bf16, tag="xin", bufs=4)
            if gt % 2 == 0:
                nc.scalar.copy(xin, xinf)
            else:
                nc.vector.tensor_copy(xin, xinf)
            xp = ps1.tile([128, 8, 128], bf16, tag="xp", bufs=3,
                          name=f"xp_{gt}")
            for dc in range(8):
                nc.tensor.transpose(
                    xp[:, dc, :], xin[:, dc * 128:(dc + 1) * 128],
                    identb)
            dst = xT[:, :, gt * 128:(gt + 1) * 128]
            if gt % 2 == 0:
                nc.vector.tensor_copy(dst, xp)
            else:
                nc.scalar.copy(dst, xp)

        # stage B: projection + QKNorm + V prep, PE-dense
        for r in range(n_ranges):
            pj = ps1.tile([128, 3, 512], f32, tag="pj", bufs=1, name=f"pj_{r}")
            for dc in range(8):
                for oc in range(3):
                    nc.tensor.matmul(
                        pj[:, oc, :],
                        wqkv_sb[:, dc, oc * 128:(oc + 1) * 128],
                        xT[:, dc, r * 512:(r + 1) * 512],
                        start=(dc == 0), stop=(dc == 7))
            sl = slice(r * 512, (r + 1) * 512)
            qTr = sbp.tile([128, 512], f32r, tag="qTr", bufs=2)
            kTr = sbp.tile([128, 512], f32r, tag="kTr", bufs=2)
            vTr = sbp.tile([128, 512], bf16, tag="vTr", bufs=2)
            nc.vector.tensor_scalar_add(qTr, pj[:, 0, :], bq_sb[:, 0:1])
            nc.vector.tensor_scalar_add(kTr, pj[:, 1, :], bq_sb[:, 1:2])
            nc.vector.tensor_scalar_add(vTr, pj[:, 2, :], bq_sb[:, 2:3])

            # QKNorm: rs = g * rsqrt(mean(t^2) + eps); q additionally folds
            # the 1/sqrt(hd) score scale.  scale/bias of the Sqrt activation
            # are per-partition host-precomputed: sqs = sqrt(sumsq*s + b),
            # rs = 1/sqs.
            for blk, blkb, gcol in ((qTr, qTb, 0), (kTr, kTb, 1)):
                sq = sbp.tile([128, 512], f32r, tag="sq", bufs=2)
                nc.scalar.activation(sq, blk, AF.Square)
                ps_r = ps1.tile([128, 512], f32, tag="psr", bufs=2,
                                name=f"psr_{r}_{gcol}")
                nc.tensor.matmul(ps_r, blkdiag, sq, start=True, stop=True)
                sqs = sbp.tile([128, 512], f32, tag="sqs", bufs=2)
                nc.scalar.activation(sqs, ps_r, AF.Sqrt,
                                     bias=gv_sb[:, 2 * gcol + 1:2 * gcol + 2],
                                     scale=gv_sb[:, 2 * gcol:2 * gcol + 1])
                rs = sbp.tile([128, 512], f32, tag="rs", bufs=2)
                nc.vector.reciprocal_approx_fast(rs, sqs)
                nc.vector.tensor_mul(blkb[:, sl], blk, rs)

            # V -> va for this range's 4 ktiles
            vp = ps1.tile([128, 8, 128], bf16, tag="xp", bufs=3,
                          name=f"vp_{r}")
            for q in range(4):
                nc.tensor.transpose(
                    vp[:, q, :], vTr[:, q * 128:(q + 1) * 128], identb)
            src = vp[:, 0:4, :].rearrange("p k (h d) -> p k h d", h=2)
            if r % 2 == 0:
                nc.scalar.copy(va[:, 4 * r:4 * r + 4, :, 0:64], src)
            else:
                nc.vector.tensor_copy(va[:, 4 * r:4 * r + 4, :, 0:64], src)

        # ================= phase 2: attention =================
        sbp_ctx.close()
        ps1_ctx.close()
        ps2_ctx = ExitStack()
        ps2 = ps2_ctx.enter_context(
            tc.tile_pool(name="ps2", bufs=1, space="PSUM"))
        sba_ctx = ExitStack()
        sba = sba_ctx.enter_context(tc.tile_pool(name="sba", bufs=1))

        def make_norm_steps(s, po):
            """Normalize stripe s + output proj, as a list of emission steps
            to be spread across the next stripe's kt loop (keeps engine
            FIFOs shallow so PE never gaps at stripe boundaries)."""
            qsl = slice(s * 512, (s + 1) * 512)
            st = {}

            def step_recip():
                # custom-DVE ops misbehave at base_partition != 0: compute
                # recip over all 65 rows from base 0; only row 64 (the
                # denominator) is consumed by the broadcast matmul
                rd = sba.tile([65, 2, 512], f32, tag="rd", bufs=2,
                              name=f"rd_{s}")
                nc.vector.reciprocal_approx_fast(rd, po)
                st["rd"] = rd

            def step_bcast():
                ps_b = ps2.tile([128, 2, 512], f32, tag="sg", bufs=2,
                                name=f"psb_{s}")
                for h in range(2):
                    nc.tensor.matmul(ps_b[0:64, h, :],
                                     ones64[64:65, :],
                                     st["rd"][64:65, h, :],
                                     start=True, stop=True,
                                     tile_position=(64, 0))
                st["psb"] = ps_b

            def step_rb():
                rb = sba.tile([64, 2, 512], f32, tag="rb", bufs=2,
                              name=f"rb_{s}")
                nc.scalar.copy(rb, st["psb"][0:64, :, :])
                st["rb"] = rb

            def step_mul():
                for h, oTn in ((0, oTn0), (1, oTn1)):
                    nc.vector.tensor_mul(oTn[:, qsl], po[0:64, h, :],
                                         st["rb"][:, h, :])

            def make_proj(tt):
                def step_proj():
                    t0 = s * 512 + tt * 128
                    ps_o = ps2.tile([128, 2, 512], f32, tag="sg", bufs=2,
                                    name=f"pso_{s}_{tt}")
                    for half in range(2):
                        nsl = slice(half * 512, (half + 1) * 512)
                        nc.tensor.matmul(ps_o[:, half, :],
                                         oTn0[:, t0:t0 + 128],
                                         wo0[:, nsl],
                                         start=True, stop=False)
                        nc.tensor.matmul(ps_o[:, half, :],
                                         oTn1[:, t0:t0 + 128],
                                         wo1[:, nsl],
                                         start=False, stop=True)
                    ob = sba.tile([128, 2, 512], f32, tag="ob", bufs=3,
                                  name=f"ob_{s}_{tt}")
                    if tt == 3:
                        nc.vector.tensor_copy(ob, ps_o)
                    else:
                        nc.scalar.copy(ob, ps_o)
                    nc.sync.dma_start(out_t[t0 // 128], ob)
                return step_proj

            return [step_recip, step_bcast, step_rb, step_mul,
                    make_proj(0), make_proj(1), make_proj(2), make_proj(3)]

        # norm steps of stripe s-1 are emitted at these kt indices of stripe s
        STEP_KTS = {2: 0, 4: 1, 6: 2, 8: 3, 10: 4, 12: 5, 14: 6, 16: 7}

        LAG = 2  # attn@V trails scores/exp by LAG ktiles so PE never waits
        pending = None
        for s in range(n_stripes):
            qsl = slice(s * 512, (s + 1) * 512)
            po = ps2.tile([65, 2, 512], f32, tag="po", bufs=2,
                          name=f"po_{s}")
            prev = []
            for kt in range(n_ktiles):
                sg = ps2.tile([128, 2, 512], f32, tag="sg", bufs=2,
                              name=f"sg_{s}_{kt}")
                for h in range(2):
                    hp = slice(h * 64, (h + 1) * 64)
                    nc.tensor.matmul(
                        sg[:, h, :],
                        kTb[hp, kt * 128:(kt + 1) * 128],
                        qTb[hp, qsl],
                        start=True, stop=True,
                        tile_position=(h * 64, 0))
                # exp (mask folded in): key frame fk vs query frames
                # (2s, 2s+1): future -> +1.0 bias; whole [128,2,512] tile by
                # one engine, alternating ScalarE (exact) / DVE (Schraudolph)
                et = sba.tile([128, 2, 512], bf16, tag="et", bufs=5,
                              name=f"et_{s}_{kt}")
                fk = kt // 2
                if kt % 2 == 0:
                    if fk == 2 * s + 1:
                        nc.scalar.activation(et[:, :, 0:256],
                                             sg[:, :, 0:256], AF.Exp,
                                             bias=1.0)
                        nc.scalar.activation(et[:, :, 256:512],
                                             sg[:, :, 256:512], AF.Exp)
                    else:
                        nc.scalar.activation(
                            et, sg, AF.Exp,
                            bias=(1.0 if fk > 2 * s + 1 else 0.0))
                else:
                    eti = et.bitcast(i16)
                    if fk == 2 * s + 1:
                        nc.vector.tensor_scalar(
                            eti[:, :, 0:256], sg[:, :, 0:256],
                            SCH_A, SCH_B + SCH_A, op0=MUL, op1=ADD)
                        nc.vector.tensor_scalar(
                            eti[:, :, 256:512], sg[:, :, 256:512],
                            SCH_A, SCH_B, op0=MUL, op1=ADD)
                    else:
                        b = SCH_B + (SCH_A if fk > 2 * s + 1 else 0.0)
                        nc.vector.tensor_scalar(
                            eti, sg, SCH_A, b, op0=MUL, op1=ADD)
                if pending is not None and kt in STEP_KTS:
                    pending[STEP_KTS[kt]]()
                    if STEP_KTS[kt] == len(pending) - 1:
                        pending = None
                prev.append((kt, et))
                if len(prev) > LAG:
                    pkt, pet = prev.pop(0)
                    for h in range(2):
                        nc.tensor.matmul(po[:, h, :],
                                         va[:, pkt, h, :],
                                         pet[:, h, :],
                                         start=(pkt == 0), stop=False)
            for pkt, pet in prev:
                for h in range(2):
                    nc.tensor.matmul(po[:, h, :], va[:, pkt, h, :],
                                     pet[:, h, :],
                                     start=False, stop=(pkt == n_ktiles - 1))
            prev = []
            pending = make_norm_steps(s, po)
        for step in pending:
            step()

        sba_ctx.close()
        ps2_ctx.close()
        ctx.close()

    nc.compile()
    return nc


def shard_inputs(x, Wqkv, bqkv, gq, gk, Wout, n_tok):
    """Build the 8 per-core input maps (head-parallel sharding)."""
    D = D_MODEL
    in_maps = []
    gq = np.asarray(gq, np.float64)
    gk = np.asarray(gk, np.float64)
    # per-partition RMS sqrt scale/bias (see build_program):
    #   q: rs = gq/8 * rsqrt(mean+eps)  -> sqs = sqrt(sumsq/gq^2 + 64eps/gq^2)
    #   k: rs = gk * rsqrt(mean+eps)    -> sqs = sqrt(sumsq/(64gk^2) + eps/gk^2)
    sq_ = np.concatenate([1.0 / gq**2, 1.0 / gq**2])
    bq_ = np.concatenate([64.0 * EPS / gq**2, 64.0 * EPS / gq**2])
    sk_ = np.concatenate([1.0 / (64.0 * gk**2), 1.0 / (64.0 * gk**2)])
    bk_ = np.concatenate([EPS / gk**2, EPS / gk**2])
    gv = np.stack([sq_, bq_, sk_, bk_], axis=1).astype(np.float32)
    for c in range(N_CORES):
        cs = slice(128 * c, 128 * (c + 1))
        wq = Wqkv[:, cs]
        wk = Wqkv[:, D + 128 * c:D + 128 * (c + 1)]
        wv = Wqkv[:, 2 * D + 128 * c:2 * D + 128 * (c + 1)]
        wqkv_s = np.ascontiguousarray(np.concatenate([wq, wk, wv], axis=1),
                                      dtype=np.float32)
        bq = bqkv[cs]
        bk = bqkv[D + 128 * c:D + 128 * (c + 1)]
        bv = bqkv[2 * D + 128 * c:2 * D + 128 * (c + 1)]
        bqkv_s = np.ascontiguousarray(np.concatenate([bq, bk, bv]),
                                      dtype=np.float32)
        wout_s = np.ascontiguousarray(Wout[cs, :], dtype=np.float32)
        in_maps.append({
            "x": np.ascontiguousarray(x[:n_tok], dtype=np.float32),
            "wqkv": wqkv_s,
            "bqkv": bqkv_s,
            "gv": np.ascontiguousarray(gv),
            "wout": wout_s,
        })
    return in_maps


_PROGRAM_CACHE = {}


def _get_program(n_tok):
    if n_tok not in _PROGRAM_CACHE:
        _PROGRAM_CACHE[n_tok] = build_program(n_tok)
    return _PROGRAM_CACHE[n_tok]


def run_sharded(inputs, trace=False, tmpdir=None):
    """Run the SPMD kernel; returns (full_output [1,N,D], BassKernelResults)."""
    from concourse.bass_utils import run_bass_kernel_spmd

    x = np.asarray(inputs["x"], dtype=np.float32)
    Wqkv = np.asarray(inputs["Wqkv"], dtype=np.float32)
    bqkv = np.asarray(inputs["bqkv"], dtype=np.float32)
    Wout = np.asarray(inputs["Wout"], dtype=np.float32)
    bout = np.asarray(inputs["bout"], dtype=np.float32)
    gq = np.asarray(inputs["gq"], dtype=np.float32)
    gk = np.asarray(inputs["gk"], dtype=np.float32)
    tpf = int(np.asarray(inputs["tokens_per_frame"]))
    assert tpf == TPF, f"kernel hardcodes tokens_per_frame={TPF}, got {tpf}"

    B, N, D = x.shape
    assert B == 1 and D == D_MODEL
    x2 = x[0]

    nc = _get_program(N)
    in_maps = shard_inputs(x2, Wqkv, bqkv, gq, gk, Wout, N)
    res = run_bass_kernel_spmd(nc, in_maps, list(range(N_CORES)),
                               trace=trace, tmpdir=tmpdir)
    acc = res.results[0]["out"].astype(np.float32)
    for c in range(1, N_CORES):
        acc = acc + res.results[c]["out"]
    if np.any(bout):
        acc = acc + bout[None, :]
    return acc[None], res


def kernel(**inputs):
    out, _ = run_sharded(inputs)
    return out
